# revision 13
# baseline (speedup 1.0000x reference)
"""Trainium2 Bass kernel for nn_AttentionBlock (B=8, H=W=32, C=512, 8 heads).

Data-parallel over batch: each of 8 NeuronCores does one batch element.

Per core, the kernel is organized around keeping ScalarE (softmax exp, the
serial floor at ~73us of ACT work) 100% busy while TensorE work rides in
its shadow:

  ramp:    x DMA (2 queues) -> PE transposes -> v (all s-tiles) ->
           q^T m=0 / k^T m=4 (head pair 0), exp-table warm-up.
  phase 2: per head h, 8 slots: S^T s-tile fill (2 K=64 matmuls) ->
           ScalarE exp(N=1024) -> same-head PV matmuls lagging one slot.
           Background qkv-projection matmuls for later head pairs are
           woven into the S^T PSUM ring (one m-tile per head).
  phase 3: out-projection directly in [t, e] layout (stationary = A^T
           chunk), DVE bias-add evacuate, DMA out.  No output transpose,
           no ScalarE copies.

Numerics identical to the proven baseline: float32r matmuls with fp32
PSUM accumulation, softmax without max-subtraction (logits ~N(0,1)),
denominators via a ones-column appended to V, host-side tf32 rounding
and v-bias folding into the output bias.
"""

import math
import os
from contextlib import ExitStack

import numpy as np

import concourse.bass as bass
import concourse.mybir as mybir
import concourse.tile as tile
from concourse import bacc

T = 1024          # tokens per batch element (32*32)
C = 512           # channels
HEADS = 8
HC = C // HEADS   # 64
P = 128           # partitions
NT = T // P       # 8 t-tiles
NCT = C // P      # 4 c-tiles
CHUNK = 512       # moving-operand chunk (fp32 max, = one PSUM bank)
NCH = T // CHUNK  # 2 chunks
F32 = mybir.dt.float32
F32R = mybir.dt.float32r
EXP_SCALE = 1.0 / math.sqrt(HC)  # (1/sqrt(sqrt(hc)))^2 applied to q·k
VW = HC + 1       # v channels + ones column


def tf32_round(a: np.ndarray) -> np.ndarray:
    """Round fp32 -> tf32 (10-bit mantissa) with round-to-nearest-even."""
    bits = a.astype(np.float32).view(np.uint32)
    round_bit = np.uint32(1 << 12)
    lsb = (bits >> np.uint32(13)) & np.uint32(1)
    bits = bits + (round_bit - np.uint32(1)) + lsb
    bits &= np.uint32(0xFFFFE000)
    return bits.view(np.float32)


def build_program(debug_dumps: bool = False):
    nc = bacc.Bacc("TRN2", num_devices=8, debug=False)

    x_d = nc.dram_tensor("x", [T, C], F32, kind="ExternalInput")
    wqkv_d = nc.dram_tensor("qkv_w", [C, 3 * C], F32R, kind="ExternalInput")
    wout_d = nc.dram_tensor("out_w", [C, C], F32R, kind="ExternalInput")
    qkb_d = nc.dram_tensor("qk_b", [2 * C], F32, kind="ExternalInput")
    ob_d = nc.dram_tensor("out_b", [C], F32, kind="ExternalInput")
    out_d = nc.dram_tensor("out", [T, C], F32, kind="ExternalOutput")

    with tile.TileContext(nc) as tc, ExitStack() as ctx:
        from concourse.masks import make_identity

        # ---------------- SBUF pools ----------------
        const = ctx.enter_context(tc.tile_pool(name="const", bufs=1))
        persist = ctx.enter_context(tc.tile_pool(name="persist", bufs=1))
        workp = ctx.enter_context(tc.tile_pool(name="workp", bufs=2))
        xin_cm = tc.tile_pool(name="xin", bufs=1)
        xin = xin_cm.__enter__()

        # x tiles: gate the transpose pipeline; split across two DMA queues
        xts = []
        for i in range(NT):
            xt_in = xin.tile([P, C], F32, tag=f"xin{i}", name=f"xin{i}")
            q = nc.sync if i % 2 == 0 else nc.scalar
            q.dma_start(xt_in[:], x_d.ap()[i * P:(i + 1) * P, :])
            xts.append(xt_in)

        # exp table warm-up: tiny activation ASAP so the ~2.7us table load
        # happens during the DMA/transpose ramp, not before the first real exp.
        warm = const.tile([1, 2], F32, tag="warm", name="warm")
        nc.gpsimd.memset(warm[:, 0:1], 0.0)
        nc.scalar.activation(warm[:, 1:2], warm[:, 0:1],
                             mybir.ActivationFunctionType.Exp)

        # weights: head-pair-0 q/k columns first, the rest on a second queue
        wq = []  # [c-tile][128, 1536] (q | k | v)
        for m in range(NCT):
            t_ = persist.tile([P, 3 * C], F32R, tag=f"wq{m}", name=f"wq{m}")
            nc.gpsimd.dma_start(t_[:, 0:640],
                                wqkv_d.ap()[m * P:(m + 1) * P, 0:640])
            wq.append(t_)
        for m in range(NCT):
            nc.gpsimd.dma_start(wq[m][:, 640:3 * C],
                                wqkv_d.ap()[m * P:(m + 1) * P, 640:3 * C])
        wo = []  # [c-tile][128, 512]
        for m in range(NCT):
            t_ = persist.tile([P, C], F32R, tag=f"wo{m}", name=f"wo{m}")
            nc.sync.dma_start(t_[:], wout_d.ap()[m * P:(m + 1) * P, :])
            wo.append(t_)

        identity = const.tile([P, P], F32, tag="ident", name="ident")
        make_identity(nc, identity[:])
        ones8 = const.tile([P, HEADS, 1], F32, tag="ones8", name="ones8")
        nc.gpsimd.memset(ones8[:], 1.0)

        # biases; column m of qkb_all = qkv_b[128m:128m+128]
        qkb_all = const.tile([P, 2 * C // P], F32, tag="qkball", name="qkb_all")
        nc.gpsimd.dma_start(
            qkb_all[:], qkb_d.ap().rearrange("(m p) -> p m", p=P)
        )
        qkb_t = [qkb_all[:, m:m + 1] for m in range(2 * C // P)]
        ob_row = const.tile([1, C], F32, tag="obrow", name="ob_row")
        nc.sync.dma_start(ob_row[:], ob_d.ap().rearrange("(o c) -> o c", o=1))
        obb = const.tile([P, C], F32, tag="obb", name="obb")
        nc.gpsimd.partition_broadcast(obb[:], ob_row[:], channels=P)

        xT = [persist.tile([P, T], F32R, tag=f"xT{m}", name=f"xT{m}")
              for m in range(NCT)]
        VAW = HEADS * VW + (P - VW)  # 128-wide lhsT reads stay in-tile
        vaug = [persist.tile([P, VAW], F32R, tag=f"va{i}", name=f"va{i}")
                for i in range(NT)]
        qkT = [persist.tile([P, T], F32R, tag=f"qk{m}", name=f"qk{m}")
               for m in range(NCT)]
        # per-head zero-padded k^T: even heads use rows 0:64 (zeros below),
        # odd heads rows 64:128, so K=128 S^T matmuls pair with full q^T rows.
        kTz = [persist.tile([P, T], F32R, tag=f"kz{h}", name=f"kz{h}")
               for h in range(HEADS)]
        anorm = [persist.tile([P, T], F32R, tag=f"an{m}", name=f"an{m}")
                 for m in range(NCT)]
        for i in range(NT):
            nc.vector.tensor_scalar_mul(
                vaug[i][:, HEADS * VW:VAW], wq[0][:, 0:VAW - HEADS * VW], 0.0
            )
        for h in range(HEADS):
            zlo = 0 if h % 2 == 1 else HC
            nc.vector.tensor_scalar_mul(
                kTz[h][zlo:zlo + HC, :], wq[0][0:HC, 0:T], 0.0
            )

        # ================= ramp: x^T, v, q^T/k^T pair 0 =================
        ps1_cm = tc.tile_pool(name="ps1", bufs=2, space="PSUM")
        ps1 = ps1_cm.__enter__()

        # x PE transpose; xT[m] = x^T rows [128m,128m+128) [c, t]
        for i in range(NT):
            ps_tr = ps1.tile([P, C], F32, tag="tr", name="ps_tr")
            for m in range(NCT):
                nc.tensor.transpose(
                    ps_tr[:, m * P:(m + 1) * P],
                    xts[i][:, m * P:(m + 1) * P],
                    identity[:],
                )
            for m in range(NCT):
                eng = nc.vector if m % 2 == 0 else nc.scalar
                if m % 2 == 0:
                    eng.tensor_copy(
                        xT[m][:, i * P:(i + 1) * P], ps_tr[:, m * P:(m + 1) * P]
                    )
                else:
                    eng.copy(
                        xT[m][:, i * P:(i + 1) * P],
                        ps_tr[:, m * P:(m + 1) * P],
                    )

        # v = x @ Wv; vaug[i]: [128(t), 8, 65], [:, h, 64] = 1.0
        for i in range(NT):
            ps_v = ps1.tile([P, C], F32, tag="v", name="ps_v")
            for m in range(NCT):
                nc.tensor.matmul(
                    ps_v[:],
                    xT[m][:, i * P:(i + 1) * P],
                    wq[m][:, 2 * C:3 * C],
                    start=(m == 0),
                    stop=(m == NCT - 1),
                )
            va3 = vaug[i][:, 0:HEADS * VW].rearrange("p (h d) -> p h d", d=VW)
            nc.vector.tensor_copy(
                va3[:, :, 0:HC],
                ps_v[:].rearrange("p (h d) -> p h d", h=HEADS),
            )
            nc.vector.tensor_copy(va3[:, :, HC:VW], ones8[:])

        def emit_qk_mtile(m, psum_pool):
            """qkv-projection m-tile (q: m<4 -> qkT[m]; k: m>=4 -> kTz pair)."""
            ps_qk = psum_pool.tile([P, 2 * CHUNK], F32, tag="st", name="ps_bg")
            for j in range(NCH):
                for cc in range(NCT):
                    nc.tensor.matmul(
                        ps_qk[:, j * CHUNK:(j + 1) * CHUNK],
                        wq[cc][:, m * P:(m + 1) * P],
                        xT[cc][:, j * CHUNK:(j + 1) * CHUNK],
                        start=(cc == 0),
                        stop=(cc == NCT - 1),
                    )
            for j in range(NCH):
                js = slice(j * CHUNK, (j + 1) * CHUNK)
                if m < NCT:
                    nc.vector.tensor_scalar_add(
                        qkT[m][:, js], ps_qk[:, js], qkb_t[m][:]
                    )
                else:
                    hh = 2 * (m - NCT)
                    nc.vector.tensor_scalar_add(
                        kTz[hh][0:HC, js], ps_qk[0:HC, js], qkb_t[m][0:HC]
                    )
                    nc.vector.tensor_scalar_add(
                        kTz[hh + 1][HC:P, js], ps_qk[HC:P, js], qkb_t[m][HC:P]
                    )

        emit_qk_mtile(0, ps1)   # q for heads 0,1
        emit_qk_mtile(4, ps1)   # k for heads 0,1
        ps1_cm.__exit__(None, None, None)
        xin_cm.__exit__(None, None, None)

        # ================= phase 2: attention =================
        # Per head: 8 slots; slot g fills S^T for s-tile g (both 512-chunks)
        # into a 2-bank PSUM tile, ScalarE exps it into exh, and the PV
        # matmuls for slot g-1 (same head) ride behind on the PE.  One
        # background qkv m-tile per head is woven into the S^T PSUM ring.
        BG_ITEMS = [1, 5, 2, 6, 3, 7, None, None]  # per-head background m-tile

        with (
            tc.tile_pool(name="expsp", bufs=4) as expsp,
            tc.tile_pool(name="ps_st", bufs=2, space="PSUM") as ps_st,
            tc.tile_pool(name="ps_pv", bufs=2, space="PSUM") as ps_pv,
        ):
            def emit_pv(h, exs, pv_ps, g):
                """PV matmuls for s-tile g of head h (exs = that slot's exp)."""
                for j in range(NCH):
                    nc.tensor.matmul(
                        pv_ps[j][:],
                        vaug[g][:, h * VW:h * VW + P],
                        exs[:, j * CHUNK:(j + 1) * CHUNK],
                        start=(g == 0),
                        stop=(g == NT - 1),
                    )

            def emit_normalize(h, pv_ps):
                aoff = (h % 2) * HC
                am = h // 2
                dtmp = workp.tile([1, T], F32, tag="dtmp", name="dtmp")
                recip = workp.tile([1, T], F32, tag="recip", name="recip")
                bcast = workp.tile([HC, T], F32, tag="bcast", name="bcast")
                for j in range(NCH):
                    js = slice(j * CHUNK, (j + 1) * CHUNK)
                    nc.vector.tensor_copy(dtmp[:, js], pv_ps[j][HC:HC + 1, :])
                    nc.vector.reciprocal_approx_fast(recip[:, js], dtmp[:, js])
                    nc.gpsimd.partition_broadcast(
                        bcast[:, js], recip[:, js], channels=HC
                    )
                    nc.vector.tensor_tensor(
                        anorm[am][aoff:aoff + HC, js],
                        pv_ps[j][0:HC, :],
                        bcast[:, js],
                        op=mybir.AluOpType.mult,
                    )

            for h in range(HEADS):
                qm = h // 2
                hlo = (h % 2) * HC
                pv_ps = [ps_pv.tile([P, CHUNK], F32, tag=f"pv{j}",
                                    name=f"pv{j}") for j in range(NCH)]
                exslots = []
                for g in range(NT):
                    st_ps = ps_st.tile([P, 2 * CHUNK], F32, tag="st", name="st")
                    for j in range(NCH):
                        nc.tensor.matmul(
                            st_ps[:, j * CHUNK:(j + 1) * CHUNK],
                            kTz[h][hlo:hlo + HC, g * P:(g + 1) * P],
                            qkT[qm][hlo:hlo + HC, j * CHUNK:(j + 1) * CHUNK],
                            start=True,
                            stop=True,
                        )
                    exs = expsp.tile([P, 2 * CHUNK], F32R, tag="exh",
                                     name="exh")
                    exslots.append(exs)
                    nc.scalar.activation(
                        exs[:],
                        st_ps[:],
                        mybir.ActivationFunctionType.Exp,
                        scale=EXP_SCALE,
                    )
                    if g >= 1:
                        emit_pv(h, exslots[g - 1], pv_ps, g - 1)
                    if g == 3 and BG_ITEMS[h] is not None:
                        emit_qk_mtile(BG_ITEMS[h], ps_st)
                emit_pv(h, exslots[NT - 1], pv_ps, NT - 1)
                emit_normalize(h, pv_ps)

        # ================= phase 3: out projection [t, e] =================
        with (
            tc.tile_pool(name="otp", bufs=2) as otp,
            tc.tile_pool(name="ps3", bufs=2, space="PSUM") as ps3,
        ):
            for i in range(NT):
                ps_o = ps3.tile([P, C], F32, tag="o", name="ps_o")
                for cc in range(NCT):
                    nc.tensor.matmul(
                        ps_o[:],
                        anorm[cc][:, i * P:(i + 1) * P],
                        wo[cc][:],
                        start=(cc == 0),
                        stop=(cc == NCT - 1),
                    )
                ot = otp.tile([P, C], F32, tag="ot", name="ot")
                nc.vector.tensor_tensor(
                    ot[:], ps_o[:], obb[:], op=mybir.AluOpType.add
                )
                nc.sync.dma_start(out_d.ap()[i * P:(i + 1) * P, :], ot[:])

    nc.compile()
    return nc


_CACHED_NC = None


def _get_nc():
    global _CACHED_NC
    if _CACHED_NC is None:
        _CACHED_NC = build_program()
    return _CACHED_NC


def kernel(x, qkv_w, qkv_b, out_w, out_b):
    """Full inputs in, full output out.  Shards batch across 8 NeuronCores."""
    from concourse.bass_utils import run_bass_kernel_spmd

    x = np.asarray(x)
    B, H, W, Cc = x.shape
    assert (B, H, W, Cc) == (8, 32, 32, C)
    x2 = np.ascontiguousarray(x.reshape(B, T, C).astype(np.float32))
    wq2 = np.asarray(qkv_w).reshape(C, 3 * C).astype(np.float32)
    wo2 = np.asarray(out_w).reshape(C, C).astype(np.float32)
    qkv_b = np.asarray(qkv_b).astype(np.float32)
    out_b = np.asarray(out_b).astype(np.float32)

    # host-side prep: tf32-round the weights (device loads them as float32r),
    # fold the v-bias through the output projection (exact: A_norm += b_v
    # shifts out by b_v @ W_out).
    wq_r = tf32_round(wq2)
    wo_r = tf32_round(wo2)
    b_v = qkv_b[2 * C:3 * C]
    ob_eff = (
        out_b.astype(np.float64) + b_v.astype(np.float64) @ wo_r.astype(np.float64)
    ).astype(np.float32)
    qkb = np.ascontiguousarray(qkv_b[0:2 * C])

    nc = _get_nc()
    in_maps = [
        {
            "x": np.ascontiguousarray(x2[b]),
            "qkv_w": np.ascontiguousarray(wq_r),
            "out_w": np.ascontiguousarray(wo_r),
            "qk_b": qkb,
            "out_b": ob_eff,
        }
        for b in range(B)
    ]
    trace = bool(int(os.environ.get("KERNEL_TRACE", "0")))
    res = run_bass_kernel_spmd(nc, in_maps, core_ids=list(range(B)), trace=trace)
    if trace and res.exec_time_ns is not None:
        print(f"HW exec time: {res.exec_time_ns} ns")
    kernel.last_results = res
    out = np.stack([res.results[b]["out"] for b in range(B)], axis=0)
    return out.reshape(B, H, W, Cc)


kernel.last_results = None


# revision 18
# speedup vs baseline: 1.0940x; 1.0940x over previous
"""Trainium2 Bass kernel for nn_AttentionBlock (B=8, H=W=32, C=512, 8 heads).

Data-parallel over batch: each of 8 NeuronCores does one batch element.

Per core, the kernel is organized around keeping ScalarE (softmax exp, the
serial floor at ~73us of ACT work) 100% busy while TensorE work rides in
its shadow:

  ramp:    x DMA (2 queues) -> PE transposes -> v (all s-tiles) ->
           q^T m=0 / k^T m=4 (head pair 0), exp-table warm-up.
  phase 2: per head h, 8 slots: S^T s-tile fill (2 K=64 matmuls) ->
           ScalarE exp(N=1024) -> same-head PV matmuls lagging one slot.
           Background qkv-projection matmuls for later head pairs are
           woven into the S^T PSUM ring (one m-tile per head).
  phase 3: out-projection directly in [t, e] layout (stationary = A^T
           chunk), DVE bias-add evacuate, DMA out.  No output transpose,
           no ScalarE copies.

Numerics identical to the proven baseline: float32r matmuls with fp32
PSUM accumulation, softmax without max-subtraction (logits ~N(0,1)),
denominators via a ones-column appended to V, host-side tf32 rounding
and v-bias folding into the output bias.
"""

import math
import os
from contextlib import ExitStack

import numpy as np

import concourse.bass as bass
import concourse.mybir as mybir
import concourse.tile as tile
from concourse import bacc

T = 1024          # tokens per batch element (32*32)
C = 512           # channels
HEADS = 8
HC = C // HEADS   # 64
P = 128           # partitions
NT = T // P       # 8 t-tiles
NCT = C // P      # 4 c-tiles
CHUNK = 512       # moving-operand chunk (fp32 max, = one PSUM bank)
NCH = T // CHUNK  # 2 chunks
F32 = mybir.dt.float32
F32R = mybir.dt.float32r
EXP_SCALE = 1.0 / math.sqrt(HC)  # (1/sqrt(sqrt(hc)))^2 applied to q·k
VW = HC + 1       # v channels + ones column


def tf32_round(a: np.ndarray) -> np.ndarray:
    """Round fp32 -> tf32 (10-bit mantissa) with round-to-nearest-even."""
    bits = a.astype(np.float32).view(np.uint32)
    round_bit = np.uint32(1 << 12)
    lsb = (bits >> np.uint32(13)) & np.uint32(1)
    bits = bits + (round_bit - np.uint32(1)) + lsb
    bits &= np.uint32(0xFFFFE000)
    return bits.view(np.float32)


def build_program(debug_dumps: bool = False):
    nc = bacc.Bacc("TRN2", num_devices=8, debug=False)

    x_d = nc.dram_tensor("x", [T, C], F32, kind="ExternalInput")
    wqkv_d = nc.dram_tensor("qkv_w", [C, 3 * C], F32R, kind="ExternalInput")
    wout_d = nc.dram_tensor("out_w", [C, C], F32R, kind="ExternalInput")
    qkb_d = nc.dram_tensor("qk_b", [2 * C], F32, kind="ExternalInput")
    ob_d = nc.dram_tensor("out_b", [C], F32, kind="ExternalInput")
    out_d = nc.dram_tensor("out", [T, C], F32, kind="ExternalOutput")

    with tile.TileContext(nc) as tc, ExitStack() as ctx:
        from concourse.masks import make_identity

        # ---------------- SBUF pools ----------------
        const = ctx.enter_context(tc.tile_pool(name="const", bufs=1))
        persist = ctx.enter_context(tc.tile_pool(name="persist", bufs=1))
        workp = ctx.enter_context(tc.tile_pool(name="workp", bufs=2))
        xin_cm = tc.tile_pool(name="xin", bufs=1)
        xin = xin_cm.__enter__()

        # x tiles: gate the transpose pipeline; split across two DMA queues
        xts = []
        for i in range(NT):
            xt_in = xin.tile([P, C], F32, tag=f"xin{i}", name=f"xin{i}")
            q = nc.sync if i % 2 == 0 else nc.scalar
            q.dma_start(xt_in[:], x_d.ap()[i * P:(i + 1) * P, :])
            xts.append(xt_in)

        # exp table warm-up: tiny activation ASAP so the ~2.7us table load
        # happens during the DMA/transpose ramp, not before the first real exp.
        warm = const.tile([1, 2], F32, tag="warm", name="warm")
        nc.gpsimd.memset(warm[:, 0:1], 0.0)
        nc.scalar.activation(warm[:, 1:2], warm[:, 0:1],
                             mybir.ActivationFunctionType.Exp)

        # gpsimd compute (identity for PE transposes) must beat the weight
        # DMA descriptor issues onto the gpsimd queue, or the first
        # transpose waits ~8us for it.
        identity = const.tile([P, P], F32, tag="ident", name="ident")
        make_identity(nc, identity[:])
        ones8 = const.tile([P, HEADS, 1], F32, tag="ones8", name="ones8")
        nc.gpsimd.memset(ones8[:], 1.0)

        # biases; column m of qkb_all = qkv_b[128m:128m+128]
        qkb_all = const.tile([P, 2 * C // P], F32, tag="qkball", name="qkb_all")
        nc.gpsimd.dma_start(
            qkb_all[:], qkb_d.ap().rearrange("(m p) -> p m", p=P)
        )
        qkb_t = [qkb_all[:, m:m + 1] for m in range(2 * C // P)]

        # weights: head-pair-0 q/k columns first, then v + remaining k
        wq = []  # [c-tile][128, 1536] (q | k | v)
        for m in range(NCT):
            t_ = persist.tile([P, 3 * C], F32R, tag=f"wq{m}", name=f"wq{m}")
            nc.gpsimd.dma_start(t_[:, 0:640],
                                wqkv_d.ap()[m * P:(m + 1) * P, 0:640])
            wq.append(t_)
        for m in range(NCT):
            nc.gpsimd.dma_start(wq[m][:, 640:3 * C],
                                wqkv_d.ap()[m * P:(m + 1) * P, 640:3 * C])
        wo = []  # [c-tile][128, 512]
        for m in range(NCT):
            t_ = persist.tile([P, C], F32R, tag=f"wo{m}", name=f"wo{m}")
            nc.sync.dma_start(t_[:], wout_d.ap()[m * P:(m + 1) * P, :])
            wo.append(t_)

        ob_row = const.tile([1, C], F32, tag="obrow", name="ob_row")
        nc.sync.dma_start(ob_row[:], ob_d.ap().rearrange("(o c) -> o c", o=1))
        obb = const.tile([P, C], F32, tag="obb", name="obb")
        nc.gpsimd.partition_broadcast(obb[:], ob_row[:], channels=P)

        xT = [persist.tile([P, T], F32R, tag=f"xT{m}", name=f"xT{m}")
              for m in range(NCT)]
        VAW = HEADS * VW + (P - VW)  # 128-wide lhsT reads stay in-tile
        vaug = [persist.tile([P, VAW], F32R, tag=f"va{i}", name=f"va{i}")
                for i in range(NT)]
        qkT = [persist.tile([P, T], F32R, tag=f"qk{m}", name=f"qk{m}")
               for m in range(NCT)]
        # compact k^T per pair: head 2p in rows 0:64, head 2p+1 in rows 64:128
        kTc = [persist.tile([P, T], F32R, tag=f"kc{m}", name=f"kc{m}")
               for m in range(NCT)]
        anorm = [persist.tile([P, T], F32R, tag=f"an{m}", name=f"an{m}")
                 for m in range(NCT)]
        for i in range(NT):
            nc.vector.tensor_scalar_mul(
                vaug[i][:, HEADS * VW:VAW], wq[0][:, 0:VAW - HEADS * VW], 0.0
            )

        # ================= ramp: x^T, v, q^T/k^T pair 0 =================
        ps1_cm = tc.tile_pool(name="ps1", bufs=2, space="PSUM")
        ps1 = ps1_cm.__enter__()

        # x PE transpose; xT[m] = x^T rows [128m,128m+128) [c, t]
        for i in range(NT):
            ps_tr = ps1.tile([P, C], F32, tag="tr", name="ps_tr")
            for m in range(NCT):
                nc.tensor.transpose(
                    ps_tr[:, m * P:(m + 1) * P],
                    xts[i][:, m * P:(m + 1) * P],
                    identity[:],
                )
            for m in range(NCT):
                eng = nc.vector if m % 2 == 0 else nc.scalar
                if m % 2 == 0:
                    eng.tensor_copy(
                        xT[m][:, i * P:(i + 1) * P], ps_tr[:, m * P:(m + 1) * P]
                    )
                else:
                    eng.copy(
                        xT[m][:, i * P:(i + 1) * P],
                        ps_tr[:, m * P:(m + 1) * P],
                    )

        def emit_qk_mtile(m, psum_pool):
            """qkv-projection m-tile (q: m<4 -> qkT[m]; k: m>=4 -> kTc)."""
            ps_qk = psum_pool.tile([P, 2 * CHUNK], F32, tag="st", name="ps_bg")
            for j in range(NCH):
                for cc in range(NCT):
                    nc.tensor.matmul(
                        ps_qk[:, j * CHUNK:(j + 1) * CHUNK],
                        wq[cc][:, m * P:(m + 1) * P],
                        xT[cc][:, j * CHUNK:(j + 1) * CHUNK],
                        start=(cc == 0),
                        stop=(cc == NCT - 1),
                    )
            dst = qkT[m] if m < NCT else kTc[m - NCT]
            for j in range(NCH):
                js = slice(j * CHUNK, (j + 1) * CHUNK)
                nc.vector.tensor_scalar_add(dst[:, js], ps_qk[:, js],
                                            qkb_t[m][:])

        def emit_v_stile(i):
            """vaug[i]: [128(t), 8, 65], [:, h, 64] = 1.0"""
            ps_v = ps1.tile([P, C], F32, tag="v", name="ps_v")
            for m in range(NCT):
                nc.tensor.matmul(
                    ps_v[:],
                    xT[m][:, i * P:(i + 1) * P],
                    wq[m][:, 2 * C:3 * C],
                    start=(m == 0),
                    stop=(m == NCT - 1),
                )
            va3 = vaug[i][:, 0:HEADS * VW].rearrange("p (h d) -> p h d", d=VW)
            nc.vector.tensor_copy(
                va3[:, :, 0:HC],
                ps_v[:].rearrange("p (h d) -> p h d", h=HEADS),
            )
            nc.vector.tensor_copy(va3[:, :, HC:VW], ones8[:])

        # pair-0 q/k first (gates the first exp), then v (consumed by PV
        # with a ~1-slot lag once phase 2 starts)
        emit_qk_mtile(0, ps1)   # q for heads 0,1
        emit_qk_mtile(4, ps1)   # k for heads 0,1
        for i in range(NT):
            emit_v_stile(i)
        ps1_cm.__exit__(None, None, None)
        xin_cm.__exit__(None, None, None)

        # ================= phase 2: attention =================
        # Per head: 8 slots; slot g fills S^T for s-tile g (both 512-chunks)
        # into a 2-bank PSUM tile, ScalarE exps it into exh, and the PV
        # matmuls for slot g-1 (same head) ride behind on the PE.  One
        # background qkv m-tile per head is woven into the S^T PSUM ring.
        BG_ITEMS = [1, 5, 2, 6, 3, 7, None, None]  # per-head background m-tile

        otp_cm = tc.tile_pool(name="otp", bufs=2)
        otp = otp_cm.__enter__()

        with (
            tc.tile_pool(name="expsp", bufs=4) as expsp,
            tc.tile_pool(name="ps_st", bufs=2, space="PSUM") as ps_st,
            tc.tile_pool(name="ps_pv", bufs=2, space="PSUM") as ps_pv,
        ):
            def emit_pv_pair(p, exs, pvt, g):
                """PV matmuls for s-tile g, both heads of pair p, one j."""
                for hh in range(2):
                    nc.tensor.matmul(
                        pvt[hh][:],
                        vaug[g][:, (2 * p + hh) * VW:(2 * p + hh) * VW + P],
                        exs[:, hh * CHUNK:(hh + 1) * CHUNK],
                        start=(g == 0),
                        stop=(g == NT - 1),
                    )

            def emit_normalize(p, hh, j, pvt):
                h = 2 * p + hh
                aoff = hh * HC
                js = slice(j * CHUNK, (j + 1) * CHUNK)
                dtmp = workp.tile([1, CHUNK], F32, tag="dtmp", name="dtmp")
                recip = workp.tile([1, CHUNK], F32, tag="recip", name="recip")
                bcast = workp.tile([HC, CHUNK], F32, tag="bcast", name="bcast")
                nc.vector.tensor_copy(dtmp[:], pvt[HC:HC + 1, :])
                nc.vector.reciprocal_approx_fast(recip[:], dtmp[:])
                nc.gpsimd.partition_broadcast(bcast[:], recip[:], channels=HC)
                nc.vector.tensor_tensor(
                    anorm[p][aoff:aoff + HC, js],
                    pvt[0:HC, :],
                    bcast[:],
                    op=mybir.AluOpType.mult,
                )

            def emit_outproj_2tiles(i0):
                """out rows [128*i0, 128*(i0+2)) via the S^T PSUM ring."""
                ps_o = ps_st.tile([P, 2 * CHUNK], F32, tag="st", name="ps_o")
                for ii in range(2):
                    i = i0 + ii
                    for cc in range(NCT):
                        nc.tensor.matmul(
                            ps_o[:, ii * CHUNK:(ii + 1) * CHUNK],
                            anorm[cc][:, i * P:(i + 1) * P],
                            wo[cc][:],
                            start=(cc == 0),
                            stop=(cc == NCT - 1),
                        )
                for ii in range(2):
                    i = i0 + ii
                    ot = otp.tile([P, C], F32, tag="ot", name="ot")
                    nc.vector.tensor_tensor(
                        ot[:], ps_o[:, ii * CHUNK:(ii + 1) * CHUNK], obb[:],
                        op=mybir.AluOpType.add,
                    )
                    nc.sync.dma_start(out_d.ap()[i * P:(i + 1) * P, :], ot[:])

            # background m-tile items per (pair, slot-index); pair 3 instead
            # pre-computes the j=0 half of the out projection in its j=1 slots
            BG = {(0, 2): 1, (0, 9): 5, (1, 2): 2, (1, 9): 6,
                  (2, 2): 3, (2, 9): 7}

            for p in range(NT // 2):
                pvt = {}
                for j in range(NCH):
                    pvt[j] = [ps_pv.tile([P, CHUNK], F32, tag=f"pv{hh}",
                                         name=f"pv{hh}") for hh in range(2)]
                    exslots = []
                    for g in range(NT):
                        st_ps = ps_st.tile([P, 2 * CHUNK], F32, tag="st",
                                           name="st")
                        # two heads' S^T run concurrently in row groups
                        # 0-1 (rows 0:64) and 2-3 (rows 64:128)
                        for hh in range(2):
                            hlo = hh * HC
                            nc.tensor.matmul(
                                st_ps[:, hh * CHUNK:(hh + 1) * CHUNK],
                                kTc[p][hlo:hlo + HC, g * P:(g + 1) * P],
                                qkT[p][hlo:hlo + HC,
                                       j * CHUNK:(j + 1) * CHUNK],
                                start=True,
                                stop=True,
                            )
                        exs = expsp.tile([P, 2 * CHUNK], F32R, tag="exh",
                                         name="exh")
                        exslots.append(exs)
                        nc.scalar.activation(
                            exs[:],
                            st_ps[:],
                            mybir.ActivationFunctionType.Exp,
                            scale=EXP_SCALE,
                        )
                        if g >= 1:
                            emit_pv_pair(p, exslots[g - 1], pvt[j], g - 1)
                        si = j * NT + g
                        if (p, si) in BG:
                            emit_qk_mtile(BG[(p, si)], ps_st)
                        elif p == 3 and si in (11, 13):
                            emit_outproj_2tiles(si - 11)
                    emit_pv_pair(p, exslots[NT - 1], pvt[j], NT - 1)
                    for hh in range(2):
                        emit_normalize(p, hh, j, pvt[j][hh])

        # ================= phase 3: out projection tail (t-tiles 4-7) =====
        with tc.tile_pool(name="ps3", bufs=2, space="PSUM") as ps3:
            for i0 in (4, 6):
                ps_o = ps3.tile([P, 2 * CHUNK], F32, tag="o", name="ps_o")
                for ii in range(2):
                    i = i0 + ii
                    for cc in range(NCT):
                        nc.tensor.matmul(
                            ps_o[:, ii * CHUNK:(ii + 1) * CHUNK],
                            anorm[cc][:, i * P:(i + 1) * P],
                            wo[cc][:],
                            start=(cc == 0),
                            stop=(cc == NCT - 1),
                        )
                for ii in range(2):
                    i = i0 + ii
                    ot = otp.tile([P, C], F32, tag="ot", name="ot")
                    nc.vector.tensor_tensor(
                        ot[:], ps_o[:, ii * CHUNK:(ii + 1) * CHUNK], obb[:],
                        op=mybir.AluOpType.add,
                    )
                    nc.sync.dma_start(out_d.ap()[i * P:(i + 1) * P, :], ot[:])
        otp_cm.__exit__(None, None, None)

    nc.compile()
    return nc


_CACHED_NC = None


def _get_nc():
    global _CACHED_NC
    if _CACHED_NC is None:
        _CACHED_NC = build_program()
    return _CACHED_NC


def kernel(x, qkv_w, qkv_b, out_w, out_b):
    """Full inputs in, full output out.  Shards batch across 8 NeuronCores."""
    from concourse.bass_utils import run_bass_kernel_spmd

    x = np.asarray(x)
    B, H, W, Cc = x.shape
    assert (B, H, W, Cc) == (8, 32, 32, C)
    x2 = np.ascontiguousarray(x.reshape(B, T, C).astype(np.float32))
    wq2 = np.asarray(qkv_w).reshape(C, 3 * C).astype(np.float32)
    wo2 = np.asarray(out_w).reshape(C, C).astype(np.float32)
    qkv_b = np.asarray(qkv_b).astype(np.float32)
    out_b = np.asarray(out_b).astype(np.float32)

    # host-side prep: tf32-round the weights (device loads them as float32r),
    # fold the v-bias through the output projection (exact: A_norm += b_v
    # shifts out by b_v @ W_out).
    wq_r = tf32_round(wq2)
    wo_r = tf32_round(wo2)
    b_v = qkv_b[2 * C:3 * C]
    ob_eff = (
        out_b.astype(np.float64) + b_v.astype(np.float64) @ wo_r.astype(np.float64)
    ).astype(np.float32)
    qkb = np.ascontiguousarray(qkv_b[0:2 * C])

    nc = _get_nc()
    in_maps = [
        {
            "x": np.ascontiguousarray(x2[b]),
            "qkv_w": np.ascontiguousarray(wq_r),
            "out_w": np.ascontiguousarray(wo_r),
            "qk_b": qkb,
            "out_b": ob_eff,
        }
        for b in range(B)
    ]
    trace = bool(int(os.environ.get("KERNEL_TRACE", "0")))
    res = run_bass_kernel_spmd(nc, in_maps, core_ids=list(range(B)), trace=trace)
    if trace and res.exec_time_ns is not None:
        print(f"HW exec time: {res.exec_time_ns} ns")
    kernel.last_results = res
    out = np.stack([res.results[b]["out"] for b in range(B)], axis=0)
    return out.reshape(B, H, W, Cc)


kernel.last_results = None


# revision 19
# speedup vs baseline: 1.1331x; 1.0357x over previous
"""Trainium2 Bass kernel for nn_AttentionBlock (B=8, H=W=32, C=512, 8 heads).

Data-parallel over batch: each of 8 NeuronCores does one batch element.

The kernel is organized around keeping ScalarE (softmax exp, the serial
floor at ~59us of ACT work) busy while TensorE work rides in its shadow:

  ramp:    x^T loaded directly via XBAR transpose-DMA (bf16), weights
           bf16 via a second queue; q^T/k^T for head pair 0; v s-tiles
           0-1; exp-table warm-up.
  phase 2: head pairs p=0..3, j-major slots (j, g): the two heads' S^T
           s-tile matmuls run concurrently in PE row groups 0-1/2-3
           (K=64 each) -> one ScalarE exp(N=1024) -> same-pair PV
           matmuls lagging one slot.  Remaining v s-tiles, later pairs'
           q^T/k^T projections, and the entire out projection are woven
           into the S^T PSUM ring as small background items.
  tail:    per t-tile 4-7: single c-tile-3 matmul + add to the SBUF
           partial -> DMA.

All matmul operands are bf16 (fp32 PSUM accumulation, fp32 softmax
denominators/reciprocals); rel err vs the fp32 reference ~2e-3, well
inside the 2e-2 gate.  Softmax without max-subtraction (logits ~N(0,1)),
denominators via a ones-column appended to V, v-bias folded into the
output bias host-side.
"""

import math
import os
from contextlib import ExitStack

import numpy as np

import concourse.bass as bass
import concourse.mybir as mybir
import concourse.tile as tile
from concourse import bacc

T = 1024          # tokens per batch element (32*32)
C = 512           # channels
HEADS = 8
HC = C // HEADS   # 64
P = 128           # partitions
NT = T // P       # 8 t-tiles
NCT = C // P      # 4 c-tiles
CHUNK = 512       # PSUM bank = 512 fp32
NCH = T // CHUNK  # 2 chunks
F32 = mybir.dt.float32
BF16 = mybir.dt.bfloat16
EXP_SCALE = 1.0 / math.sqrt(HC)
VW = HC + 1       # v channels + ones column
NPAIR = HEADS // 2


def build_program(debug_dumps: bool = False):
    nc = bacc.Bacc("TRN2", num_devices=8, debug=False)

    x_d = nc.dram_tensor("x", [T, C], BF16, kind="ExternalInput")
    wqkv_d = nc.dram_tensor("qkv_w", [C, 3 * C], BF16, kind="ExternalInput")
    wout_d = nc.dram_tensor("out_w", [C, C], BF16, kind="ExternalInput")
    qkb_d = nc.dram_tensor("qk_b", [2 * C], F32, kind="ExternalInput")
    ob_d = nc.dram_tensor("out_b", [C], F32, kind="ExternalInput")
    out_d = nc.dram_tensor("out", [T, C], F32, kind="ExternalOutput")

    with tile.TileContext(nc) as tc, ExitStack() as ctx:
        # ---------------- SBUF pools ----------------
        const = ctx.enter_context(tc.tile_pool(name="const", bufs=1))
        persist = ctx.enter_context(tc.tile_pool(name="persist", bufs=1))
        workp = ctx.enter_context(tc.tile_pool(name="workp", bufs=2))
        otp = ctx.enter_context(tc.tile_pool(name="otp", bufs=2))

        # exp table warm-up ASAP (the ~2.7us table load hides under DMA)
        warm = const.tile([1, 2], F32, tag="warm", name="warm")
        nc.gpsimd.memset(warm[:, 0:1], 0.0)
        nc.scalar.activation(warm[:, 1:2], warm[:, 0:1],
                             mybir.ActivationFunctionType.Exp)

        # x^T via XBAR transpose-DMA, one per c-tile, split across queues
        xT = [persist.tile([P, T], BF16, tag=f"xT{m}", name=f"xT{m}")
              for m in range(NCT)]
        for m in range(NCT):
            q = nc.sync if m % 2 == 0 else nc.scalar
            q.dma_start_transpose(xT[m][:], x_d.ap()[:, m * P:(m + 1) * P])

        # biases; column m of qkb_all = qkv_b[128m:128m+128]
        qkb_all = const.tile([P, 2 * C // P], F32, tag="qkball", name="qkb_all")
        nc.gpsimd.dma_start(
            qkb_all[:], qkb_d.ap().rearrange("(m p) -> p m", p=P)
        )
        qkb_t = [qkb_all[:, m:m + 1] for m in range(2 * C // P)]

        # weights: pair-0 q/k columns first
        wq = []  # [c-tile][128, 1536] (q | k | v)
        for m in range(NCT):
            t_ = persist.tile([P, 3 * C], BF16, tag=f"wq{m}", name=f"wq{m}")
            nc.gpsimd.dma_start(t_[:, 0:640],
                                wqkv_d.ap()[m * P:(m + 1) * P, 0:640])
            wq.append(t_)
        for m in range(NCT):
            nc.gpsimd.dma_start(wq[m][:, 640:3 * C],
                                wqkv_d.ap()[m * P:(m + 1) * P, 640:3 * C])
        wo = []  # [c-tile][128, 512]
        for m in range(NCT):
            t_ = persist.tile([P, C], BF16, tag=f"wo{m}", name=f"wo{m}")
            nc.sync.dma_start(t_[:], wout_d.ap()[m * P:(m + 1) * P, :])
            wo.append(t_)

        ones8 = const.tile([P, HEADS, 1], F32, tag="ones8", name="ones8")
        nc.gpsimd.memset(ones8[:], 1.0)
        ob_row = const.tile([1, C], F32, tag="obrow", name="ob_row")
        nc.sync.dma_start(ob_row[:], ob_d.ap().rearrange("(o c) -> o c", o=1))
        obb = const.tile([P, C], F32, tag="obb", name="obb")
        nc.gpsimd.partition_broadcast(obb[:], ob_row[:], channels=P)

        VAW = HEADS * VW + (P - VW)  # 128-wide lhsT reads stay in-tile
        vaug = [persist.tile([P, VAW], BF16, tag=f"va{i}", name=f"va{i}")
                for i in range(NT)]
        qkT = [persist.tile([P, T], BF16, tag=f"qk{m}", name=f"qk{m}")
               for m in range(NCT)]
        # compact k^T per pair: head 2p in rows 0:64, head 2p+1 in 64:128
        kTc = [persist.tile([P, T], BF16, tag=f"kc{m}", name=f"kc{m}")
               for m in range(NCT)]
        anorm = [persist.tile([P, T], BF16, tag=f"an{m}", name=f"an{m}")
                 for m in range(NCT)]
        # fp32 out-projection partials (c-tiles 0-2 + bias) for t-tiles 4-7
        partp = ctx.enter_context(tc.tile_pool(name="partp", bufs=1))
        part = [partp.tile([P, C], F32, tag=f"pt{i}", name=f"pt{i}")
                for i in range(4)]
        for i in range(NT):
            nc.vector.tensor_scalar_mul(
                vaug[i][:, HEADS * VW:VAW], wq[0][:, 0:VAW - HEADS * VW], 0.0
            )

        # ================= background item emitters =================
        def emit_qk_half(m, j, pool):
            """qkv-projection m-tile, chunk j (q: m<4 -> qkT; k: -> kTc)."""
            ps_qk = pool.tile([P, 2 * CHUNK], F32, tag="st", name="ps_bg")
            js = slice(j * CHUNK, (j + 1) * CHUNK)
            for cc in range(NCT):
                nc.tensor.matmul(
                    ps_qk[:, 0:CHUNK],
                    wq[cc][:, m * P:(m + 1) * P],
                    xT[cc][:, js],
                    start=(cc == 0),
                    stop=(cc == NCT - 1),
                )
            dst = qkT[m] if m < NCT else kTc[m - NCT]
            nc.vector.tensor_scalar_add(dst[:, js], ps_qk[:, 0:CHUNK],
                                        qkb_t[m][:])

        def emit_v_2tiles(i0, pool):
            """vaug[i0], vaug[i0+1]: [128(t), 8, 65], [:, h, 64] = 1.0"""
            ps_v = pool.tile([P, 2 * CHUNK], F32, tag="st", name="ps_v")
            for ii in range(2):
                i = i0 + ii
                for m in range(NCT):
                    nc.tensor.matmul(
                        ps_v[:, ii * CHUNK:(ii + 1) * CHUNK],
                        xT[m][:, i * P:(i + 1) * P],
                        wq[m][:, 2 * C:3 * C],
                        start=(m == 0),
                        stop=(m == NCT - 1),
                    )
            for ii in range(2):
                i = i0 + ii
                va3 = vaug[i][:, 0:HEADS * VW].rearrange(
                    "p (h d) -> p h d", d=VW)
                nc.vector.tensor_copy(
                    va3[:, :, 0:HC],
                    ps_v[:, ii * CHUNK:(ii + 1) * CHUNK].rearrange(
                        "p (h d) -> p h d", h=HEADS),
                )
                nc.vector.tensor_copy(va3[:, :, HC:VW], ones8[:])

        def emit_outproj_partial(i0, pool):
            """part[i-4] = out-proj c-tiles 0-2 + bias for t-tiles i0, i0+1."""
            ps_o = pool.tile([P, 2 * CHUNK], F32, tag="st", name="ps_op")
            for ii in range(2):
                i = i0 + ii
                for cc in range(NCT - 1):
                    nc.tensor.matmul(
                        ps_o[:, ii * CHUNK:(ii + 1) * CHUNK],
                        anorm[cc][:, i * P:(i + 1) * P],
                        wo[cc][:],
                        start=(cc == 0),
                        stop=(cc == NCT - 2),
                    )
            for ii in range(2):
                i = i0 + ii
                nc.vector.tensor_tensor(
                    part[i - 4][:], ps_o[:, ii * CHUNK:(ii + 1) * CHUNK],
                    obb[:], op=mybir.AluOpType.add,
                )

        def emit_outproj_full(i0, pool):
            """out rows [128*i0, 128*(i0+2)): all c-tiles + bias -> DMA."""
            ps_o = pool.tile([P, 2 * CHUNK], F32, tag="st", name="ps_of")
            for ii in range(2):
                i = i0 + ii
                for cc in range(NCT):
                    nc.tensor.matmul(
                        ps_o[:, ii * CHUNK:(ii + 1) * CHUNK],
                        anorm[cc][:, i * P:(i + 1) * P],
                        wo[cc][:],
                        start=(cc == 0),
                        stop=(cc == NCT - 1),
                    )
            for ii in range(2):
                i = i0 + ii
                ot = otp.tile([P, C], F32, tag="ot", name="ot")
                nc.vector.tensor_tensor(
                    ot[:], ps_o[:, ii * CHUNK:(ii + 1) * CHUNK], obb[:],
                    op=mybir.AluOpType.add,
                )
                nc.sync.dma_start(out_d.ap()[i * P:(i + 1) * P, :], ot[:])

        # ================= ramp =================
        ps1_cm = tc.tile_pool(name="ps1", bufs=2, space="PSUM")
        ps1 = ps1_cm.__enter__()
        for j in range(NCH):
            emit_qk_half(0, j, ps1)         # q pair 0
        for j in range(NCH):
            emit_qk_half(NCT, j, ps1)       # k pair 0
        emit_v_2tiles(0, ps1)               # v s-tiles 0,1
        ps1_cm.__exit__(None, None, None)

        # ================= phase 2: attention (head pairs) =================
        # weave schedule: (p, j, g) -> emitter run after that slot
        WEAVE = {
            (0, 0, 1): lambda pool: emit_v_2tiles(2, pool),
            (0, 0, 3): lambda pool: emit_v_2tiles(4, pool),
            (0, 0, 5): lambda pool: emit_v_2tiles(6, pool),
            (0, 1, 1): lambda pool: emit_qk_half(1, 0, pool),
            (0, 1, 3): lambda pool: emit_qk_half(1, 1, pool),
            (0, 1, 5): lambda pool: emit_qk_half(NCT + 1, 0, pool),
            (0, 1, 6): lambda pool: emit_qk_half(NCT + 1, 1, pool),
            (1, 0, 2): lambda pool: emit_qk_half(2, 0, pool),
            (1, 0, 5): lambda pool: emit_qk_half(2, 1, pool),
            (1, 1, 2): lambda pool: emit_qk_half(NCT + 2, 0, pool),
            (1, 1, 5): lambda pool: emit_qk_half(NCT + 2, 1, pool),
            (2, 0, 2): lambda pool: emit_qk_half(3, 0, pool),
            (2, 0, 5): lambda pool: emit_qk_half(3, 1, pool),
            (2, 1, 2): lambda pool: emit_qk_half(NCT + 3, 0, pool),
            (2, 1, 5): lambda pool: emit_qk_half(NCT + 3, 1, pool),
            (3, 0, 2): lambda pool: emit_outproj_partial(4, pool),
            (3, 0, 5): lambda pool: emit_outproj_partial(6, pool),
            (3, 1, 3): lambda pool: emit_outproj_full(0, pool),
            (3, 1, 5): lambda pool: emit_outproj_full(2, pool),
        }

        with (
            tc.tile_pool(name="expsp", bufs=4) as expsp,
            tc.tile_pool(name="ps_st", bufs=2, space="PSUM") as ps_st,
            tc.tile_pool(name="ps_pv", bufs=2, space="PSUM") as ps_pv,
        ):
            def emit_pv_pair(p, exs, pvt, g):
                """PV matmuls for s-tile g, both heads of pair p, one j."""
                for hh in range(2):
                    nc.tensor.matmul(
                        pvt[hh][:],
                        vaug[g][:, (2 * p + hh) * VW:(2 * p + hh) * VW + P],
                        exs[:, hh * CHUNK:(hh + 1) * CHUNK],
                        start=(g == 0),
                        stop=(g == NT - 1),
                    )

            def emit_normalize(p, hh, j, pvt):
                aoff = hh * HC
                js = slice(j * CHUNK, (j + 1) * CHUNK)
                dtmp = workp.tile([1, CHUNK], F32, tag="dtmp", name="dtmp")
                recip = workp.tile([1, CHUNK], F32, tag="recip", name="recip")
                bcast = workp.tile([HC, CHUNK], F32, tag="bcast", name="bcast")
                nc.vector.tensor_copy(dtmp[:], pvt[HC:HC + 1, :])
                nc.vector.reciprocal_approx_fast(recip[:], dtmp[:])
                nc.gpsimd.partition_broadcast(bcast[:], recip[:], channels=HC)
                nc.vector.tensor_tensor(
                    anorm[p][aoff:aoff + HC, js],
                    pvt[0:HC, :],
                    bcast[:],
                    op=mybir.AluOpType.mult,
                )

            for p in range(NPAIR):
                pvt = {}
                for j in range(NCH):
                    pvt[j] = [ps_pv.tile([P, CHUNK], F32, tag=f"pv{hh}",
                                         name=f"pv{hh}") for hh in range(2)]
                    exslots = []
                    for g in range(NT):
                        st_ps = ps_st.tile([P, 2 * CHUNK], F32, tag="st",
                                           name="st")
                        # two heads' S^T concurrently in row groups 0-1/2-3
                        for hh in range(2):
                            hlo = hh * HC
                            nc.tensor.matmul(
                                st_ps[:, hh * CHUNK:(hh + 1) * CHUNK],
                                kTc[p][hlo:hlo + HC, g * P:(g + 1) * P],
                                qkT[p][hlo:hlo + HC,
                                       j * CHUNK:(j + 1) * CHUNK],
                                start=True,
                                stop=True,
                            )
                        exs = expsp.tile([P, 2 * CHUNK], BF16, tag="exh",
                                         name="exh")
                        exslots.append(exs)
                        nc.scalar.activation(
                            exs[:],
                            st_ps[:],
                            mybir.ActivationFunctionType.Exp,
                            scale=EXP_SCALE,
                        )
                        if g >= 1:
                            emit_pv_pair(p, exslots[g - 1], pvt[j], g - 1)
                        if (p, j, g) in WEAVE:
                            WEAVE[(p, j, g)](ps_st)
                    emit_pv_pair(p, exslots[NT - 1], pvt[j], NT - 1)
                    for hh in range(2):
                        emit_normalize(p, hh, j, pvt[j][hh])

                if p == NPAIR - 1:
                    # tail: t-tiles 4-7 need only the last c-tile matmul on
                    # top of the SBUF partial; ride the freed pv ring banks
                    for i in range(4, NT):
                        ps_o = ps_pv.tile([P, CHUNK], F32, tag=f"pv{i % 2}",
                                          name="ps_tl")
                        nc.tensor.matmul(
                            ps_o[:],
                            anorm[NCT - 1][:, i * P:(i + 1) * P],
                            wo[NCT - 1][:],
                            start=True,
                            stop=True,
                        )
                        ot = otp.tile([P, C], F32, tag="ot", name="ot")
                        nc.vector.tensor_tensor(
                            ot[:], ps_o[:], part[i - 4][:],
                            op=mybir.AluOpType.add,
                        )
                        nc.sync.dma_start(
                            out_d.ap()[i * P:(i + 1) * P, :], ot[:]
                        )

    nc.compile()
    return nc


def host_prep(x, qkv_w, qkv_b, out_w, out_b):
    """Host-side input prep shared by kernel() and the sim harness."""
    import ml_dtypes

    x = np.asarray(x)
    B = x.shape[0]
    x2 = x.reshape(B, T, C).astype(np.float32)
    wq2 = np.asarray(qkv_w).reshape(C, 3 * C).astype(np.float32)
    wo2 = np.asarray(out_w).reshape(C, C).astype(np.float32)
    qkv_b = np.asarray(qkv_b).astype(np.float32)
    out_b = np.asarray(out_b).astype(np.float32)

    bf = ml_dtypes.bfloat16
    x_bf = x2.astype(bf)
    wq_bf = wq2.astype(bf)
    wo_bf = wo2.astype(bf)
    # fold the v-bias through the output projection (A_norm += b_v shifts
    # out by b_v @ W_out)
    b_v = qkv_b[2 * C:3 * C]
    ob_eff = (
        out_b.astype(np.float64)
        + b_v.astype(np.float64) @ wo_bf.astype(np.float64)
    ).astype(np.float32)
    qkb = np.ascontiguousarray(qkv_b[0:2 * C])
    return x_bf, wq_bf, wo_bf, qkb, ob_eff


_CACHED_NC = None


def _get_nc():
    global _CACHED_NC
    if _CACHED_NC is None:
        _CACHED_NC = build_program()
    return _CACHED_NC


def kernel(x, qkv_w, qkv_b, out_w, out_b):
    """Full inputs in, full output out.  Shards batch across 8 NeuronCores."""
    from concourse.bass_utils import run_bass_kernel_spmd

    x = np.asarray(x)
    B, H, W, Cc = x.shape
    assert (B, H, W, Cc) == (8, 32, 32, C)
    x_bf, wq_bf, wo_bf, qkb, ob_eff = host_prep(x, qkv_w, qkv_b, out_w, out_b)

    nc = _get_nc()
    in_maps = [
        {
            "x": np.ascontiguousarray(x_bf[b]),
            "qkv_w": np.ascontiguousarray(wq_bf),
            "out_w": np.ascontiguousarray(wo_bf),
            "qk_b": qkb,
            "out_b": ob_eff,
        }
        for b in range(B)
    ]
    trace = bool(int(os.environ.get("KERNEL_TRACE", "0")))
    res = run_bass_kernel_spmd(nc, in_maps, core_ids=list(range(B)), trace=trace)
    if trace and res.exec_time_ns is not None:
        print(f"HW exec time: {res.exec_time_ns} ns")
    kernel.last_results = res
    out = np.stack([res.results[b]["out"] for b in range(B)], axis=0)
    return out.reshape(B, H, W, Cc)


kernel.last_results = None


# revision 26
# speedup vs baseline: 1.1808x; 1.0421x over previous
"""Trainium2 Bass kernel for nn_AttentionBlock (B=8, H=W=32, C=512, 8 heads).

Data-parallel over batch: each of 8 NeuronCores does one batch element.

The kernel is organized around keeping ScalarE (softmax exp, the serial
floor at ~59us of ACT work) busy while TensorE work rides in its shadow:

  ramp:    x^T loaded directly via XBAR transpose-DMA (bf16), weights
           bf16 via a second queue; q^T/k^T for head pair 0; v s-tiles
           0-1; exp-table warm-up.
  phase 2: head pairs p=0..3, j-major slots (j, g): the two heads' S^T
           s-tile matmuls run concurrently in PE row groups 0-1/2-3
           (K=64 each) -> one ScalarE exp(N=1024) -> same-pair PV
           matmuls lagging one slot.  Remaining v s-tiles, later pairs'
           q^T/k^T projections, and the entire out projection are woven
           into the S^T PSUM ring as small background items.
  tail:    per t-tile 4-7: single c-tile-3 matmul + add to the SBUF
           partial -> DMA.

All matmul operands are bf16 (fp32 PSUM accumulation, fp32 softmax
denominators/reciprocals); rel err vs the fp32 reference ~2e-3, well
inside the 2e-2 gate.  Softmax without max-subtraction (logits ~N(0,1)),
denominators via a ones-column appended to V, v-bias folded into the
output bias host-side.
"""

import math
import os
from contextlib import ExitStack

import numpy as np

import concourse.bass as bass
import concourse.mybir as mybir
import concourse.tile as tile
from concourse import bacc

T = 1024          # tokens per batch element (32*32)
C = 512           # channels
HEADS = 8
HC = C // HEADS   # 64
P = 128           # partitions
NT = T // P       # 8 t-tiles
NCT = C // P      # 4 c-tiles
CHUNK = 512       # PSUM bank = 512 fp32
NCH = T // CHUNK  # 2 chunks
F32 = mybir.dt.float32
BF16 = mybir.dt.bfloat16
EXP_SCALE = 1.0 / math.sqrt(HC)
VW = HC + 1       # v channels + ones column
NPAIR = HEADS // 2


def build_program(debug_dumps: bool = False):
    nc = bacc.Bacc("TRN2", num_devices=8, debug=False)

    x_d = nc.dram_tensor("x", [T, C], BF16, kind="ExternalInput")
    wqkv_d = nc.dram_tensor("qkv_w", [C, 3 * C], BF16, kind="ExternalInput")
    wout_d = nc.dram_tensor("out_w", [C, C], BF16, kind="ExternalInput")
    qkb_d = nc.dram_tensor("qk_b", [2 * C], F32, kind="ExternalInput")
    ob_d = nc.dram_tensor("out_b", [C], F32, kind="ExternalInput")
    out_d = nc.dram_tensor("out", [T, C], F32, kind="ExternalOutput")

    with tile.TileContext(nc) as tc, ExitStack() as ctx:
        # ---------------- SBUF pools ----------------
        const = ctx.enter_context(tc.tile_pool(name="const", bufs=1))
        persist = ctx.enter_context(tc.tile_pool(name="persist", bufs=1))
        workp = ctx.enter_context(tc.tile_pool(name="workp", bufs=2))
        otp = ctx.enter_context(tc.tile_pool(name="otp", bufs=2))
        partp = ctx.enter_context(tc.tile_pool(name="partp", bufs=1))

        # exp table warm-up ASAP (the ~2.7us table load hides under DMA)
        warm = const.tile([1, 2], F32, tag="warm", name="warm")
        nc.gpsimd.memset(warm[:, 0:1], 0.0)
        nc.scalar.activation(warm[:, 1:2], warm[:, 0:1],
                             mybir.ActivationFunctionType.Exp)

        # x tiles then PE transposes (XBAR transpose-DMA measured ~8us/tile
        # -- far slower than the PE path)
        xT = [persist.tile([P, T], BF16, tag=f"xT{m}", name=f"xT{m}")
              for m in range(NCT)]
        xin_cm = tc.tile_pool(name="xin", bufs=1)
        xin = xin_cm.__enter__()
        xts = []
        for i in range(NT):
            xt_in = xin.tile([P, C], BF16, tag=f"xin{i}", name=f"xin{i}")
            q = nc.sync if i % 2 == 0 else nc.scalar
            q.dma_start(xt_in[:], x_d.ap()[i * P:(i + 1) * P, :])
            xts.append(xt_in)
        identity = const.tile([P, P], BF16, tag="ident", name="ident")
        from concourse.masks import make_identity
        make_identity(nc, identity[:])

        # biases; column m of qkb_all = qkv_b[128m:128m+128]
        qkb_all = const.tile([P, 2 * C // P], F32, tag="qkball", name="qkb_all")
        nc.gpsimd.dma_start(
            qkb_all[:], qkb_d.ap().rearrange("(m p) -> p m", p=P)
        )
        qkb_t = [qkb_all[:, m:m + 1] for m in range(2 * C // P)]

        # weights: pair-0 q/k columns first
        wq = []  # [c-tile][128, 1536] (q | k | v)
        for m in range(NCT):
            t_ = persist.tile([P, 3 * C], BF16, tag=f"wq{m}", name=f"wq{m}")
            nc.gpsimd.dma_start(t_[:, 0:640],
                                wqkv_d.ap()[m * P:(m + 1) * P, 0:640])
            wq.append(t_)
        for m in range(NCT):
            nc.gpsimd.dma_start(wq[m][:, 640:3 * C],
                                wqkv_d.ap()[m * P:(m + 1) * P, 640:3 * C])
        wo = []  # [c-tile][128, 512]
        for m in range(NCT):
            t_ = persist.tile([P, C], BF16, tag=f"wo{m}", name=f"wo{m}")
            nc.sync.dma_start(t_[:], wout_d.ap()[m * P:(m + 1) * P, :])
            wo.append(t_)

        ones8 = const.tile([P, HEADS, 1], F32, tag="ones8", name="ones8")
        nc.gpsimd.memset(ones8[:], 1.0)
        ob_row = const.tile([1, C], F32, tag="obrow", name="ob_row")
        nc.sync.dma_start(ob_row[:], ob_d.ap().rearrange("(o c) -> o c", o=1))
        obb = const.tile([P, C], F32, tag="obb", name="obb")
        nc.gpsimd.partition_broadcast(obb[:], ob_row[:], channels=P)

        VAW = HEADS * VW + (P - VW)  # 128-wide lhsT reads stay in-tile
        vaug = [persist.tile([P, VAW], BF16, tag=f"va{i}", name=f"va{i}")
                for i in range(NT)]
        qkT = [persist.tile([P, T], BF16, tag=f"qk{m}", name=f"qk{m}")
               for m in range(NCT)]
        # compact k^T per pair: head 2p in rows 0:64, head 2p+1 in 64:128
        kTc = [persist.tile([P, T], BF16, tag=f"kc{m}", name=f"kc{m}")
               for m in range(NCT)]
        anorm = [persist.tile([P, T], BF16, tag=f"an{m}", name=f"an{m}")
                 for m in range(NCT)]
        # fp32 out-projection partials (c-tiles 0-2 + bias) for t-tiles 4-7
        part = [partp.tile([P, C], F32, tag=f"pt{i}", name=f"pt{i}")
                for i in range(4)]
        for i in range(NT):
            nc.vector.tensor_scalar_mul(
                vaug[i][:, HEADS * VW:VAW], wq[0][:, 0:VAW - HEADS * VW], 0.0
            )

        # ================= background item emitters =================
        def emit_qk_half(m, j, pool):
            """qkv-projection m-tile, chunk j (q: m<4 -> qkT; k: -> kTc)."""
            ps_qk = pool.tile([P, 2 * CHUNK], F32, tag="st", name="ps_bg")
            js = slice(j * CHUNK, (j + 1) * CHUNK)
            for cc in range(NCT):
                nc.tensor.matmul(
                    ps_qk[:, 0:CHUNK],
                    wq[cc][:, m * P:(m + 1) * P],
                    xT[cc][:, js],
                    start=(cc == 0),
                    stop=(cc == NCT - 1),
                )
            dst = qkT[m] if m < NCT else kTc[m - NCT]
            nc.vector.tensor_scalar_add(dst[:, js], ps_qk[:, 0:CHUNK],
                                        qkb_t[m][:])

        def emit_v_2tiles(i0, pool):
            """vaug[i0], vaug[i0+1]: [128(t), 8, 65], [:, h, 64] = 1.0"""
            ps_v = pool.tile([P, 2 * CHUNK], F32, tag="st", name="ps_v")
            for ii in range(2):
                i = i0 + ii
                for m in range(NCT):
                    nc.tensor.matmul(
                        ps_v[:, ii * CHUNK:(ii + 1) * CHUNK],
                        xT[m][:, i * P:(i + 1) * P],
                        wq[m][:, 2 * C:3 * C],
                        start=(m == 0),
                        stop=(m == NCT - 1),
                    )
            for ii in range(2):
                i = i0 + ii
                va3 = vaug[i][:, 0:HEADS * VW].rearrange(
                    "p (h d) -> p h d", d=VW)
                nc.vector.tensor_copy(
                    va3[:, :, 0:HC],
                    ps_v[:, ii * CHUNK:(ii + 1) * CHUNK].rearrange(
                        "p (h d) -> p h d", h=HEADS),
                )
                nc.vector.tensor_copy(va3[:, :, HC:VW], ones8[:])

        def emit_outproj_partial(i0, pool):
            """part[i-4] = out-proj c-tiles 0-2 + bias for t-tiles i0, i0+1."""
            ps_o = pool.tile([P, 2 * CHUNK], F32, tag="st", name="ps_op")
            for ii in range(2):
                i = i0 + ii
                for cc in range(NCT - 1):
                    nc.tensor.matmul(
                        ps_o[:, ii * CHUNK:(ii + 1) * CHUNK],
                        anorm[cc][:, i * P:(i + 1) * P],
                        wo[cc][:],
                        start=(cc == 0),
                        stop=(cc == NCT - 2),
                    )
            for ii in range(2):
                i = i0 + ii
                nc.vector.tensor_tensor(
                    part[i - 4][:], ps_o[:, ii * CHUNK:(ii + 1) * CHUNK],
                    obb[:], op=mybir.AluOpType.add,
                )

        def emit_outproj_full(i0, pool):
            """out rows [128*i0, 128*(i0+2)): all c-tiles + bias -> DMA."""
            ps_o = pool.tile([P, 2 * CHUNK], F32, tag="st", name="ps_of")
            for ii in range(2):
                i = i0 + ii
                for cc in range(NCT):
                    nc.tensor.matmul(
                        ps_o[:, ii * CHUNK:(ii + 1) * CHUNK],
                        anorm[cc][:, i * P:(i + 1) * P],
                        wo[cc][:],
                        start=(cc == 0),
                        stop=(cc == NCT - 1),
                    )
            for ii in range(2):
                i = i0 + ii
                ot = otp.tile([P, C], F32, tag="ot", name="ot")
                nc.vector.tensor_tensor(
                    ot[:], ps_o[:, ii * CHUNK:(ii + 1) * CHUNK], obb[:],
                    op=mybir.AluOpType.add,
                )
                nc.sync.dma_start(out_d.ap()[i * P:(i + 1) * P, :], ot[:])

        # ================= ramp =================
        ps1_cm = tc.tile_pool(name="ps1", bufs=2, space="PSUM")
        ps1 = ps1_cm.__enter__()
        # x PE transpose (bf16); xT[m] = x^T rows [128m,128m+128) [c, t]
        for i in range(NT):
            ps_tr = ps1.tile([P, C], BF16, tag="tr", name="ps_tr")
            for m in range(NCT):
                nc.tensor.transpose(
                    ps_tr[:, m * P:(m + 1) * P],
                    xts[i][:, m * P:(m + 1) * P],
                    identity[:],
                )
            for m in range(NCT):
                nc.vector.tensor_copy(
                    xT[m][:, i * P:(i + 1) * P], ps_tr[:, m * P:(m + 1) * P]
                )
        for j in range(NCH):
            emit_qk_half(0, j, ps1)         # q pair 0
        for j in range(NCH):
            emit_qk_half(NCT, j, ps1)       # k pair 0
        emit_v_2tiles(0, ps1)               # v s-tiles 0,1
        ps1_cm.__exit__(None, None, None)
        xin_cm.__exit__(None, None, None)

        # ================= phase 2: attention (head pairs) =================
        # weave schedule: (p, j, g) -> emitter run after that slot
        WEAVE = {
            (0, 0, 1): lambda pool: emit_v_2tiles(2, pool),
            (0, 0, 3): lambda pool: emit_v_2tiles(4, pool),
            (0, 0, 5): lambda pool: emit_v_2tiles(6, pool),
            (0, 1, 1): lambda pool: emit_qk_half(1, 0, pool),
            (0, 1, 3): lambda pool: emit_qk_half(1, 1, pool),
            (0, 1, 5): lambda pool: emit_qk_half(NCT + 1, 0, pool),
            (0, 1, 6): lambda pool: emit_qk_half(NCT + 1, 1, pool),
            (1, 0, 2): lambda pool: emit_qk_half(2, 0, pool),
            (1, 0, 5): lambda pool: emit_qk_half(2, 1, pool),
            (1, 1, 2): lambda pool: emit_qk_half(NCT + 2, 0, pool),
            (1, 1, 5): lambda pool: emit_qk_half(NCT + 2, 1, pool),
            (2, 0, 2): lambda pool: emit_qk_half(3, 0, pool),
            (2, 0, 5): lambda pool: emit_qk_half(3, 1, pool),
            (2, 1, 2): lambda pool: emit_qk_half(NCT + 3, 0, pool),
            (2, 1, 5): lambda pool: emit_qk_half(NCT + 3, 1, pool),
            (3, 0, 2): lambda pool: emit_outproj_partial(4, pool),
            (3, 0, 5): lambda pool: emit_outproj_partial(6, pool),
            (3, 1, 3): lambda pool: emit_outproj_full(0, pool),
            (3, 1, 5): lambda pool: emit_outproj_full(2, pool),
        }

        with (
            tc.tile_pool(name="expsp", bufs=4) as expsp,
            tc.tile_pool(name="ps_st", bufs=2, space="PSUM") as ps_st,
            tc.tile_pool(name="ps_pv", bufs=2, space="PSUM") as ps_pv,
        ):
            def emit_pv_pair(p, exs, pvt, g):
                """PV matmuls for s-tile g, both heads of pair p, one j."""
                for hh in range(2):
                    nc.tensor.matmul(
                        pvt[:, hh * CHUNK:(hh + 1) * CHUNK],
                        vaug[g][:, (2 * p + hh) * VW:(2 * p + hh) * VW + P],
                        exs[:, hh * CHUNK:(hh + 1) * CHUNK],
                        start=(g == 0),
                        stop=(g == NT - 1),
                    )

            def emit_normalize(p, j, pvt):
                """Scale both heads' PV by 1/denominator -> anorm[p]."""
                js = slice(j * CHUNK, (j + 1) * CHUNK)
                dtmp = workp.tile([1, 2 * CHUNK], F32, tag="dtmp", name="dtmp")
                recip = workp.tile([1, 2 * CHUNK], F32, tag="recip",
                                   name="recip")
                bcast = workp.tile([HC, 2 * CHUNK], F32, tag="bcast",
                                   name="bcast")
                nc.vector.tensor_copy(dtmp[:], pvt[HC:HC + 1, :])
                nc.vector.reciprocal_approx_fast(recip[:], dtmp[:])
                nc.gpsimd.partition_broadcast(bcast[:], recip[:], channels=HC)
                for hh in range(2):
                    nc.vector.tensor_tensor(
                        anorm[p][hh * HC:(hh + 1) * HC, js],
                        pvt[0:HC, hh * CHUNK:(hh + 1) * CHUNK],
                        bcast[:, hh * CHUNK:(hh + 1) * CHUNK],
                        op=mybir.AluOpType.mult,
                    )

            for p in range(NPAIR):
                pvt = {}
                for j in range(NCH):
                    pvt[j] = ps_pv.tile([P, 2 * CHUNK], F32, tag="pv",
                                        name="pv")
                    exslots = []
                    for g in range(NT):
                        st_ps = ps_st.tile([P, 2 * CHUNK], F32, tag="st",
                                           name="st")
                        # two heads' S^T concurrently in row groups 0-1/2-3
                        for hh in range(2):
                            hlo = hh * HC
                            nc.tensor.matmul(
                                st_ps[:, hh * CHUNK:(hh + 1) * CHUNK],
                                kTc[p][hlo:hlo + HC, g * P:(g + 1) * P],
                                qkT[p][hlo:hlo + HC,
                                       j * CHUNK:(j + 1) * CHUNK],
                                start=True,
                                stop=True,
                            )
                        exs = expsp.tile([P, 2 * CHUNK], BF16, tag="exh",
                                         name="exh")
                        exslots.append(exs)
                        nc.scalar.activation(
                            exs[:],
                            st_ps[:],
                            mybir.ActivationFunctionType.Exp,
                            scale=EXP_SCALE,
                        )
                        if g >= 1:
                            emit_pv_pair(p, exslots[g - 1], pvt[j], g - 1)
                        if (p, j, g) in WEAVE:
                            WEAVE[(p, j, g)](ps_st)
                    emit_pv_pair(p, exslots[NT - 1], pvt[j], NT - 1)
                    emit_normalize(p, j, pvt[j])

                if p == NPAIR - 1:
                    # tail: t-tiles 4-7 need only the last c-tile matmul on
                    # top of the SBUF partial; ride the freed pv ring banks
                    for i in range(4, NT, 2):
                        ps_o2 = ps_pv.tile([P, 2 * CHUNK], F32, tag="pv",
                                           name="ps_tl")
                        for ii in range(2):
                            nc.tensor.matmul(
                                ps_o2[:, ii * CHUNK:(ii + 1) * CHUNK],
                                anorm[NCT - 1][:, (i + ii) * P:
                                               (i + ii + 1) * P],
                                wo[NCT - 1][:],
                                start=True,
                                stop=True,
                            )
                        for ii in range(2):
                            ot = otp.tile([P, C], F32, tag="ot", name="ot")
                            nc.vector.tensor_tensor(
                                ot[:],
                                ps_o2[:, ii * CHUNK:(ii + 1) * CHUNK],
                                part[i + ii - 4][:],
                                op=mybir.AluOpType.add,
                            )
                            nc.sync.dma_start(
                                out_d.ap()[(i + ii) * P:(i + ii + 1) * P, :],
                                ot[:],
                            )

    nc.compile()
    return nc


def host_prep(x, qkv_w, qkv_b, out_w, out_b):
    """Host-side input prep shared by kernel() and the sim harness."""
    import ml_dtypes

    x = np.asarray(x)
    B = x.shape[0]
    x2 = x.reshape(B, T, C).astype(np.float32)
    wq2 = np.asarray(qkv_w).reshape(C, 3 * C).astype(np.float32)
    wo2 = np.asarray(out_w).reshape(C, C).astype(np.float32)
    qkv_b = np.asarray(qkv_b).astype(np.float32)
    out_b = np.asarray(out_b).astype(np.float32)

    bf = ml_dtypes.bfloat16
    x_bf = x2.astype(bf)
    wq_bf = wq2.astype(bf)
    wo_bf = wo2.astype(bf)
    # fold the v-bias through the output projection (A_norm += b_v shifts
    # out by b_v @ W_out)
    b_v = qkv_b[2 * C:3 * C]
    ob_eff = (
        out_b.astype(np.float64)
        + b_v.astype(np.float64) @ wo_bf.astype(np.float64)
    ).astype(np.float32)
    qkb = np.ascontiguousarray(qkv_b[0:2 * C])
    return x_bf, wq_bf, wo_bf, qkb, ob_eff


_CACHED_NC = None


def _get_nc():
    global _CACHED_NC
    if _CACHED_NC is None:
        _CACHED_NC = build_program()
    return _CACHED_NC


def kernel(x, qkv_w, qkv_b, out_w, out_b):
    """Full inputs in, full output out.  Shards batch across 8 NeuronCores."""
    from concourse.bass_utils import run_bass_kernel_spmd

    x = np.asarray(x)
    B, H, W, Cc = x.shape
    assert (B, H, W, Cc) == (8, 32, 32, C)
    x_bf, wq_bf, wo_bf, qkb, ob_eff = host_prep(x, qkv_w, qkv_b, out_w, out_b)

    nc = _get_nc()
    in_maps = [
        {
            "x": np.ascontiguousarray(x_bf[b]),
            "qkv_w": np.ascontiguousarray(wq_bf),
            "out_w": np.ascontiguousarray(wo_bf),
            "qk_b": qkb,
            "out_b": ob_eff,
        }
        for b in range(B)
    ]
    trace = bool(int(os.environ.get("KERNEL_TRACE", "0")))
    res = run_bass_kernel_spmd(nc, in_maps, core_ids=list(range(B)), trace=trace)
    if trace and res.exec_time_ns is not None:
        print(f"HW exec time: {res.exec_time_ns} ns")
    kernel.last_results = res
    out = np.stack([res.results[b]["out"] for b in range(B)], axis=0)
    return out.reshape(B, H, W, Cc)


kernel.last_results = None


# revision 31
# speedup vs baseline: 1.1888x; 1.0068x over previous
"""Trainium2 Bass kernel for nn_AttentionBlock (B=8, H=W=32, C=512, 8 heads).

Data-parallel over batch: each of 8 NeuronCores does one batch element.

The kernel is organized around keeping ScalarE (softmax exp, the serial
floor at ~59us of ACT work) busy while TensorE work rides in its shadow:

  ramp:    x^T loaded directly via XBAR transpose-DMA (bf16), weights
           bf16 via a second queue; q^T/k^T for head pair 0; v s-tiles
           0-1; exp-table warm-up.
  phase 2: head pairs p=0..3, j-major slots (j, g): the two heads' S^T
           s-tile matmuls run concurrently in PE row groups 0-1/2-3
           (K=64 each) -> one ScalarE exp(N=1024) -> same-pair PV
           matmuls lagging one slot.  Remaining v s-tiles, later pairs'
           q^T/k^T projections, and the entire out projection are woven
           into the S^T PSUM ring as small background items.
  tail:    per t-tile 4-7: single c-tile-3 matmul + add to the SBUF
           partial -> DMA.

All matmul operands are bf16 (fp32 PSUM accumulation, fp32 softmax
denominators/reciprocals); rel err vs the fp32 reference ~2e-3, well
inside the 2e-2 gate.  Softmax without max-subtraction (logits ~N(0,1)),
denominators via a ones-column appended to V, v-bias folded into the
output bias host-side.
"""

import math
import os
from contextlib import ExitStack

import numpy as np

import concourse.bass as bass
import concourse.mybir as mybir
import concourse.tile as tile
from concourse import bacc

T = 1024          # tokens per batch element (32*32)
C = 512           # channels
HEADS = 8
HC = C // HEADS   # 64
P = 128           # partitions
NT = T // P       # 8 t-tiles
NCT = C // P      # 4 c-tiles
CHUNK = 512       # PSUM bank = 512 fp32
NCH = T // CHUNK  # 2 chunks
F32 = mybir.dt.float32
BF16 = mybir.dt.bfloat16
EXP_SCALE = 1.0 / math.sqrt(HC)
VW = HC + 1       # v channels + ones column
NPAIR = HEADS // 2


def build_program(debug_dumps: bool = False):
    nc = bacc.Bacc("TRN2", num_devices=8, debug=False)

    x_d = nc.dram_tensor("x", [T, C], BF16, kind="ExternalInput")
    wqkv_d = nc.dram_tensor("qkv_w", [C, 3 * C], BF16, kind="ExternalInput")
    wout_d = nc.dram_tensor("out_w", [C, C], BF16, kind="ExternalInput")
    qkb_d = nc.dram_tensor("qk_b", [2 * C], F32, kind="ExternalInput")
    ob_d = nc.dram_tensor("out_b", [C], F32, kind="ExternalInput")
    out_d = nc.dram_tensor("out", [T, C], F32, kind="ExternalOutput")

    with tile.TileContext(nc) as tc, ExitStack() as ctx:
        # ---------------- SBUF pools ----------------
        const = ctx.enter_context(tc.tile_pool(name="const", bufs=1))
        persist = ctx.enter_context(tc.tile_pool(name="persist", bufs=1))
        workp = ctx.enter_context(tc.tile_pool(name="workp", bufs=2))
        otp = ctx.enter_context(tc.tile_pool(name="otp", bufs=2))
        partp = ctx.enter_context(tc.tile_pool(name="partp", bufs=1))

        # exp table warm-up ASAP (the ~2.7us table load hides under DMA)
        warm = const.tile([1, 2], F32, tag="warm", name="warm")
        nc.gpsimd.memset(warm[:, 0:1], 0.0)
        nc.scalar.activation(warm[:, 1:2], warm[:, 0:1],
                             mybir.ActivationFunctionType.Exp)

        # x tiles then PE transposes (XBAR transpose-DMA measured ~8us/tile
        # -- far slower than the PE path)
        xT = [persist.tile([P, T], BF16, tag=f"xT{m}", name=f"xT{m}")
              for m in range(NCT)]
        identity = const.tile([P, P], BF16, tag="ident", name="ident")
        from concourse.masks import make_identity
        make_identity(nc, identity[:])
        xin_cm = tc.tile_pool(name="xin", bufs=1)
        xin = xin_cm.__enter__()
        xts = []
        for i in range(NT):
            xt_in = xin.tile([P, C], BF16, tag=f"xin{i}", name=f"xin{i}")
            q = (nc.sync, nc.scalar, nc.gpsimd)[i % 3]
            q.dma_start(xt_in[:], x_d.ap()[i * P:(i + 1) * P, :])
            xts.append(xt_in)

        # biases; column m of qkb_all = qkv_b[128m:128m+128]
        qkb_all = const.tile([P, 2 * C // P], F32, tag="qkball", name="qkb_all")
        nc.gpsimd.dma_start(
            qkb_all[:], qkb_d.ap().rearrange("(m p) -> p m", p=P)
        )
        qkb_t = [qkb_all[:, m:m + 1] for m in range(2 * C // P)]

        # weights: pair-0 q/k columns first
        wq = []  # [c-tile][128, 1536] (q | k | v)
        for m in range(NCT):
            t_ = persist.tile([P, 3 * C], BF16, tag=f"wq{m}", name=f"wq{m}")
            nc.gpsimd.dma_start(t_[:, 0:640],
                                wqkv_d.ap()[m * P:(m + 1) * P, 0:640])
            wq.append(t_)
        for m in range(NCT):
            nc.gpsimd.dma_start(wq[m][:, 640:3 * C],
                                wqkv_d.ap()[m * P:(m + 1) * P, 640:3 * C])
        wo = []  # [c-tile][128, 512]
        for m in range(NCT):
            t_ = persist.tile([P, C], BF16, tag=f"wo{m}", name=f"wo{m}")
            nc.sync.dma_start(t_[:], wout_d.ap()[m * P:(m + 1) * P, :])
            wo.append(t_)

        ones8 = const.tile([P, HEADS, 1], F32, tag="ones8", name="ones8")
        nc.gpsimd.memset(ones8[:], 1.0)
        ob_row = const.tile([1, C], F32, tag="obrow", name="ob_row")
        nc.sync.dma_start(ob_row[:], ob_d.ap().rearrange("(o c) -> o c", o=1))
        obb = const.tile([P, C], F32, tag="obb", name="obb")
        nc.gpsimd.partition_broadcast(obb[:], ob_row[:], channels=P)

        VAW = HEADS * VW + (P - VW)  # 128-wide lhsT reads stay in-tile
        vaug = [persist.tile([P, VAW], BF16, tag=f"va{i}", name=f"va{i}")
                for i in range(NT)]
        qkT = [persist.tile([P, T], BF16, tag=f"qk{m}", name=f"qk{m}")
               for m in range(NCT)]
        # compact k^T per pair: head 2p in rows 0:64, head 2p+1 in 64:128
        kTc = [persist.tile([P, T], BF16, tag=f"kc{m}", name=f"kc{m}")
               for m in range(NCT)]
        anorm = [persist.tile([P, T], BF16, tag=f"an{m}", name=f"an{m}")
                 for m in range(NCT)]
        # fp32 out-projection partials (c-tiles 0-2 + bias) for t-tiles 4-7
        part = [partp.tile([P, C], F32, tag=f"pt{i}", name=f"pt{i}")
                for i in range(4)]
        for i in range(NT):
            nc.vector.tensor_scalar_mul(
                vaug[i][:, HEADS * VW:VAW], wq[0][:, 0:VAW - HEADS * VW], 0.0
            )

        # ================= background item emitters =================
        def emit_qk_half(m, j, pool):
            """qkv-projection m-tile, chunk j (q: m<4 -> qkT; k: -> kTc)."""
            ps_qk = pool.tile([P, 2 * CHUNK], F32, tag="st", name="ps_bg")
            js = slice(j * CHUNK, (j + 1) * CHUNK)
            for cc in range(NCT):
                nc.tensor.matmul(
                    ps_qk[:, 0:CHUNK],
                    wq[cc][:, m * P:(m + 1) * P],
                    xT[cc][:, js],
                    start=(cc == 0),
                    stop=(cc == NCT - 1),
                )
            dst = qkT[m] if m < NCT else kTc[m - NCT]
            nc.vector.tensor_scalar_add(dst[:, js], ps_qk[:, 0:CHUNK],
                                        qkb_t[m][:])

        def emit_v_2tiles(i0, pool):
            """vaug[i0], vaug[i0+1]: [128(t), 8, 65], [:, h, 64] = 1.0"""
            ps_v = pool.tile([P, 2 * CHUNK], F32, tag="st", name="ps_v")
            for ii in range(2):
                i = i0 + ii
                for m in range(NCT):
                    nc.tensor.matmul(
                        ps_v[:, ii * CHUNK:(ii + 1) * CHUNK],
                        xT[m][:, i * P:(i + 1) * P],
                        wq[m][:, 2 * C:3 * C],
                        start=(m == 0),
                        stop=(m == NCT - 1),
                    )
            for ii in range(2):
                i = i0 + ii
                va3 = vaug[i][:, 0:HEADS * VW].rearrange(
                    "p (h d) -> p h d", d=VW)
                nc.vector.tensor_copy(
                    va3[:, :, 0:HC],
                    ps_v[:, ii * CHUNK:(ii + 1) * CHUNK].rearrange(
                        "p (h d) -> p h d", h=HEADS),
                )
                nc.vector.tensor_copy(va3[:, :, HC:VW], ones8[:])

        def emit_outproj_partial(i, pool):
            """part[i-4] = out-proj c-tiles 0-2 + bias for t-tile i."""
            ps_o = pool.tile([P, 2 * CHUNK], F32, tag="st", name="ps_op")
            for cc in range(NCT - 1):
                nc.tensor.matmul(
                    ps_o[:, 0:CHUNK],
                    anorm[cc][:, i * P:(i + 1) * P],
                    wo[cc][:],
                    start=(cc == 0),
                    stop=(cc == NCT - 2),
                )
            nc.vector.tensor_tensor(
                part[i - 4][:], ps_o[:, 0:CHUNK], obb[:],
                op=mybir.AluOpType.add,
            )

        def emit_outproj_full(i, pool):
            """out rows [128*i, 128*(i+1)): all c-tiles + bias -> DMA."""
            ps_o = pool.tile([P, 2 * CHUNK], F32, tag="st", name="ps_of")
            for cc in range(NCT):
                nc.tensor.matmul(
                    ps_o[:, 0:CHUNK],
                    anorm[cc][:, i * P:(i + 1) * P],
                    wo[cc][:],
                    start=(cc == 0),
                    stop=(cc == NCT - 1),
                )
            ot = otp.tile([P, C], F32, tag="ot", name="ot")
            nc.vector.tensor_tensor(
                ot[:], ps_o[:, 0:CHUNK], obb[:], op=mybir.AluOpType.add,
            )
            nc.sync.dma_start(out_d.ap()[i * P:(i + 1) * P, :], ot[:])

        # ================= ramp =================
        ps1_cm = tc.tile_pool(name="ps1", bufs=2, space="PSUM")
        ps1 = ps1_cm.__enter__()
        # x PE transpose (bf16); xT[m] = x^T rows [128m,128m+128) [c, t]
        for i in range(NT):
            ps_tr = ps1.tile([P, C], BF16, tag="tr", name="ps_tr")
            for m in range(NCT):
                nc.tensor.transpose(
                    ps_tr[:, m * P:(m + 1) * P],
                    xts[i][:, m * P:(m + 1) * P],
                    identity[:],
                )
            for m in range(NCT):
                nc.vector.tensor_copy(
                    xT[m][:, i * P:(i + 1) * P], ps_tr[:, m * P:(m + 1) * P]
                )
        for j in range(NCH):
            emit_qk_half(0, j, ps1)         # q pair 0
        for j in range(NCH):
            emit_qk_half(NCT, j, ps1)       # k pair 0
        emit_v_2tiles(0, ps1)               # v s-tiles 0,1
        ps1_cm.__exit__(None, None, None)
        xin_cm.__exit__(None, None, None)

        # ================= phase 2: attention (head pairs) =================
        # weave schedule: (p, j, g) -> emitter run after that slot
        WEAVE = {
            (0, 0, 1): lambda pool: emit_v_2tiles(2, pool),
            (0, 0, 3): lambda pool: emit_v_2tiles(4, pool),
            (0, 0, 5): lambda pool: emit_v_2tiles(6, pool),
            (0, 1, 1): lambda pool: emit_qk_half(1, 0, pool),
            (0, 1, 3): lambda pool: emit_qk_half(1, 1, pool),
            (0, 1, 5): lambda pool: emit_qk_half(NCT + 1, 0, pool),
            (0, 1, 6): lambda pool: emit_qk_half(NCT + 1, 1, pool),
            (1, 0, 2): lambda pool: emit_qk_half(2, 0, pool),
            (1, 0, 5): lambda pool: emit_qk_half(2, 1, pool),
            (1, 1, 2): lambda pool: emit_qk_half(NCT + 2, 0, pool),
            (1, 1, 5): lambda pool: emit_qk_half(NCT + 2, 1, pool),
            (2, 0, 2): lambda pool: emit_qk_half(3, 0, pool),
            (2, 0, 5): lambda pool: emit_qk_half(3, 1, pool),
            (2, 1, 2): lambda pool: emit_qk_half(NCT + 3, 0, pool),
            (2, 1, 5): lambda pool: emit_qk_half(NCT + 3, 1, pool),
            (3, 0, 1): lambda pool: emit_outproj_partial(4, pool),
            (3, 0, 3): lambda pool: emit_outproj_partial(5, pool),
            (3, 0, 5): lambda pool: emit_outproj_partial(6, pool),
            (3, 0, 6): lambda pool: emit_outproj_partial(7, pool),
            (3, 1, 3): lambda pool: emit_outproj_full(0, pool),
            (3, 1, 4): lambda pool: emit_outproj_full(1, pool),
            (3, 1, 5): lambda pool: emit_outproj_full(2, pool),
            (3, 1, 6): lambda pool: emit_outproj_full(3, pool),
        }

        with (
            tc.tile_pool(name="expsp", bufs=4) as expsp,
            tc.tile_pool(name="ps_st", bufs=2, space="PSUM") as ps_st,
            tc.tile_pool(name="ps_pv", bufs=2, space="PSUM") as ps_pv,
        ):
            def emit_pv_pair(p, exs, pvt, g):
                """PV matmuls for s-tile g, both heads of pair p, one j."""
                for hh in range(2):
                    nc.tensor.matmul(
                        pvt[:, hh * CHUNK:(hh + 1) * CHUNK],
                        vaug[g][:, (2 * p + hh) * VW:(2 * p + hh) * VW + P],
                        exs[:, hh * CHUNK:(hh + 1) * CHUNK],
                        start=(g == 0),
                        stop=(g == NT - 1),
                    )

            def emit_normalize(p, j, pvt):
                """Scale both heads' PV by 1/denominator -> anorm[p]."""
                js = slice(j * CHUNK, (j + 1) * CHUNK)
                # NB: reciprocal_approx_fast reading PSUM directly returns
                # garbage on HW (sim-only OK) -- stage through SBUF.
                dtmp = workp.tile([1, 2 * CHUNK], F32, tag="dtmp", name="dtmp")
                recip = workp.tile([1, 2 * CHUNK], F32, tag="recip",
                                   name="recip")
                bcast = workp.tile([HC, 2 * CHUNK], F32, tag="bcast",
                                   name="bcast")
                nc.vector.tensor_copy(dtmp[:], pvt[HC:HC + 1, :])
                nc.vector.reciprocal_approx_fast(recip[:], dtmp[:])
                nc.gpsimd.partition_broadcast(bcast[:], recip[:], channels=HC)
                for hh in range(2):
                    nc.vector.tensor_tensor(
                        anorm[p][hh * HC:(hh + 1) * HC, js],
                        pvt[0:HC, hh * CHUNK:(hh + 1) * CHUNK],
                        bcast[:, hh * CHUNK:(hh + 1) * CHUNK],
                        op=mybir.AluOpType.mult,
                    )

            for p in range(NPAIR):
                pvt = {}
                for j in range(NCH):
                    pvt[j] = ps_pv.tile([P, 2 * CHUNK], F32, tag="pv",
                                        name="pv")
                    exslots = []
                    for g in range(NT):
                        st_ps = ps_st.tile([P, 2 * CHUNK], F32, tag="st",
                                           name="st")
                        # two heads' S^T concurrently in row groups 0-1/2-3
                        for hh in range(2):
                            hlo = hh * HC
                            nc.tensor.matmul(
                                st_ps[:, hh * CHUNK:(hh + 1) * CHUNK],
                                kTc[p][hlo:hlo + HC, g * P:(g + 1) * P],
                                qkT[p][hlo:hlo + HC,
                                       j * CHUNK:(j + 1) * CHUNK],
                                start=True,
                                stop=True,
                            )
                        exs = expsp.tile([P, 2 * CHUNK], BF16, tag="exh",
                                         name="exh")
                        exslots.append(exs)
                        nc.scalar.activation(
                            exs[:],
                            st_ps[:],
                            mybir.ActivationFunctionType.Exp,
                            scale=EXP_SCALE,
                        )
                        if g >= 1:
                            emit_pv_pair(p, exslots[g - 1], pvt[j], g - 1)
                        if (p, j, g) in WEAVE:
                            WEAVE[(p, j, g)](ps_st)
                    emit_pv_pair(p, exslots[NT - 1], pvt[j], NT - 1)
                    emit_normalize(p, j, pvt[j])

                if p == NPAIR - 1:
                    # tail: t-tiles 4-7 need only the last c-tile matmul on
                    # top of the SBUF partial; ride the freed pv ring banks
                    for i in range(4, NT, 2):
                        ps_o2 = ps_pv.tile([P, 2 * CHUNK], F32, tag="pv",
                                           name="ps_tl")
                        for ii in range(2):
                            nc.tensor.matmul(
                                ps_o2[:, ii * CHUNK:(ii + 1) * CHUNK],
                                anorm[NCT - 1][:, (i + ii) * P:
                                               (i + ii + 1) * P],
                                wo[NCT - 1][:],
                                start=True,
                                stop=True,
                            )
                        for ii in range(2):
                            ot = otp.tile([P, C], F32, tag="ot", name="ot")
                            nc.vector.tensor_tensor(
                                ot[:],
                                ps_o2[:, ii * CHUNK:(ii + 1) * CHUNK],
                                part[i + ii - 4][:],
                                op=mybir.AluOpType.add,
                            )
                            nc.sync.dma_start(
                                out_d.ap()[(i + ii) * P:(i + ii + 1) * P, :],
                                ot[:],
                            )

    nc.compile()
    return nc


def host_prep(x, qkv_w, qkv_b, out_w, out_b):
    """Host-side input prep shared by kernel() and the sim harness."""
    import ml_dtypes

    x = np.asarray(x)
    B = x.shape[0]
    x2 = x.reshape(B, T, C).astype(np.float32)
    wq2 = np.asarray(qkv_w).reshape(C, 3 * C).astype(np.float32)
    wo2 = np.asarray(out_w).reshape(C, C).astype(np.float32)
    qkv_b = np.asarray(qkv_b).astype(np.float32)
    out_b = np.asarray(out_b).astype(np.float32)

    bf = ml_dtypes.bfloat16
    x_bf = x2.astype(bf)
    wq_bf = wq2.astype(bf)
    wo_bf = wo2.astype(bf)
    # fold the v-bias through the output projection (A_norm += b_v shifts
    # out by b_v @ W_out)
    b_v = qkv_b[2 * C:3 * C]
    ob_eff = (
        out_b.astype(np.float64)
        + b_v.astype(np.float64) @ wo_bf.astype(np.float64)
    ).astype(np.float32)
    qkb = np.ascontiguousarray(qkv_b[0:2 * C])
    return x_bf, wq_bf, wo_bf, qkb, ob_eff


_CACHED_NC = None


def _get_nc():
    global _CACHED_NC
    if _CACHED_NC is None:
        _CACHED_NC = build_program()
    return _CACHED_NC


def kernel(x, qkv_w, qkv_b, out_w, out_b):
    """Full inputs in, full output out.  Shards batch across 8 NeuronCores."""
    from concourse.bass_utils import run_bass_kernel_spmd

    x = np.asarray(x)
    B, H, W, Cc = x.shape
    assert (B, H, W, Cc) == (8, 32, 32, C)
    x_bf, wq_bf, wo_bf, qkb, ob_eff = host_prep(x, qkv_w, qkv_b, out_w, out_b)

    nc = _get_nc()
    in_maps = [
        {
            "x": np.ascontiguousarray(x_bf[b]),
            "qkv_w": np.ascontiguousarray(wq_bf),
            "out_w": np.ascontiguousarray(wo_bf),
            "qk_b": qkb,
            "out_b": ob_eff,
        }
        for b in range(B)
    ]
    trace = bool(int(os.environ.get("KERNEL_TRACE", "0")))
    res = run_bass_kernel_spmd(nc, in_maps, core_ids=list(range(B)), trace=trace)
    if trace and res.exec_time_ns is not None:
        print(f"HW exec time: {res.exec_time_ns} ns")
    kernel.last_results = res
    out = np.stack([res.results[b]["out"] for b in range(B)], axis=0)
    return out.reshape(B, H, W, Cc)


kernel.last_results = None
